# revision 1
# baseline (speedup 1.0000x reference)
"""Trainium2 Bass kernel for nn_Bert_Proj_CRF (BERT projection + CRF NLL).

Strategy (data-parallel over batch, 8 NeuronCores x 8 sequences):
  - Embedding rows are gathered straight into x^T layout (d on partitions)
    with a bf16 transpose-gather (dma_gather), so the projection matmul
    out[s,t] = x @ (shared_W + domain_A[corpus_b]) needs no on-chip transpose.
  - log-softmax over T=4 on chip -> LS (log-probs) and q (probs).
  - CRF normalizer: the forward scan is an ordered product of 4x4 matrices
    M_t = exp(trans) * q_t (identity where masked).  We compute it as a
    chunked associative product: 64 chunks x 8 steps per sequence run in
    parallel across 512 lanes (linear space, rescaled), then a tree combine.
  - Gold path score via one-hot selections + ones-vector matmul reductions.
"""

import numpy as np
import ml_dtypes

import concourse.bass as bass
import concourse.bacc as bacc
import concourse.tile as tile
import concourse.mybir as mybir

V, D, T = 21128, 768, 4
B, S = 64, 512
NCORES = 8
BL = B // NCORES            # 8 sequences per core
NTOK = BL * S               # 4096 tokens per core
NG = NTOK // 128            # 32 token groups of 128
F32 = mybir.dt.float32
BF16 = mybir.dt.bfloat16
I16 = mybir.dt.int16
AF = mybir.ActivationFunctionType
AL = mybir.AluOpType
AX = mybir.AxisListType


def fap(t, off, dims):
    """AP over tile t's partition dim with custom free dims (element units)."""
    base = t if isinstance(t, bass.AP) else t[:]
    return bass.AP(
        tensor=base.tensor,
        offset=base.offset + off,
        ap=[list(base.ap[0])] + [list(d) for d in dims],
    )


def dap(handle, ap):
    return bass.AP(tensor=handle, offset=0, ap=[list(d) for d in ap])


_CACHE = {}
import os
KV = os.environ.get('KV', '')


def _build():
    if "nc" in _CACHE:
        return _CACHE["nc"]
    nc = bacc.Bacc()

    table_h = nc.dram_tensor("table", [V, D], BF16, kind="ExternalInput")
    gidx_h = nc.dram_tensor("gidx", [128, NG * 8], I16, kind="ExternalInput")
    hostf_h = nc.dram_tensor("hostf", [7, 128, NG], F32, kind="ExternalInput")
    hostp_h = nc.dram_tensor("hostp", [176], F32, kind="ExternalInput")
    a8t_h = nc.dram_tensor("a8t", [D, BL * T], F32, kind="ExternalInput")
    sw_h = nc.dram_tensor("sw", [D, T], F32, kind="ExternalInput")
    nll_h = nc.dram_tensor("nll", [BL], F32, kind="ExternalOutput")
    qd_h = nc.dram_tensor("qscratch", [NTOK * T], F32, kind="Internal")
    pzd_h = nc.dram_tensor("pzscratch", [128 * 21], F32, kind="Internal")
    gd_h = nc.dram_tensor("gscratch", [BL], F32, kind="Internal")

    with tile.TileContext(nc) as tc:
        with (
            tc.tile_pool(name="consts", bufs=1) as cp,
            tc.tile_pool(name="xt", bufs=3) as xp,
            tc.tile_pool(name="work", bufs=1) as wp,
            tc.tile_pool(name="psum", bufs=1, space="PSUM") as pp,
            tc.tile_pool(name="psum2", bufs=1, space="PSUM") as pp2,
        ):
            # ---------------- constants / inputs ----------------
            hostc = cp.tile([128, 7, NG], F32)
            nc.sync.dma_start(
                out=hostc[:],
                in_=dap(hostf_h, [[NG, 128], [128 * NG, 7], [1, NG]]),
            )
            words_mm = hostc[:, 0, :]
            target_mm = hostc[:, 1, :]
            tprev_mm = hostc[:, 2, :]
            sfirst_mm = hostc[:, 3, :]
            s0sel_mm = hostc[:, 4, :]
            words_sc = hostc[:, 5, :]
            sfirst_sc = hostc[:, 6, :]

            pc = cp.tile([128, 176], F32)
            nc.gpsimd.dma_start(out=pc[:], in_=dap(hostp_h, [[0, 128], [1, 176]]))
            db_bc = fap(pc, 0, [[1, 128]])            # [(b,s_hi,j)] = 128
            sb_bc = fap(pc, 128, [[1, 4]])
            trans_bc = fap(pc, 132, [[1, 16]])
            start_bc = fap(pc, 148, [[1, 4]])
            end_bc = fap(pc, 152, [[1, 4]])

            gidx = cp.tile([128, NG * 8], I16)
            nc.sync.dma_start(out=gidx[:], in_=gidx_h[:])

            a8t = cp.tile([128, 6, BL * T], F32)
            nc.sync.dma_start(
                out=a8t[:], in_=a8t_h[:].rearrange("(c p) j -> p c j", p=128)
            )
            swt = cp.tile([128, 6, T], F32)
            nc.sync.dma_start(
                out=swt[:], in_=sw_h[:].rearrange("(c p) j -> p c j", p=128)
            )
            w8f = cp.tile([128, 6, BL * T], F32)
            nc.vector.tensor_tensor(
                out=w8f[:],
                in0=a8t[:],
                in1=fap(swt, 0, [[4, 6], [0, BL], [1, T]]),
                op=AL.add,
            )
            w8b = cp.tile([128, 6, BL * T], BF16)
            nc.gpsimd.tensor_copy(out=w8b[:], in_=w8f[:])

            expT_bc = cp.tile([128, 16], F32)
            nc.scalar.activation(out=expT_bc[:], in_=trans_bc, func=AF.Exp)
            expS = cp.tile([128, T], F32)
            nc.scalar.activation(out=expS[:], in_=start_bc, func=AF.Exp)
            expE = cp.tile([128, T], F32)
            nc.scalar.activation(out=expE[:], in_=end_bc, func=AF.Exp)

            ones128 = cp.tile([128, 1], F32)
            nc.vector.memset(ones128[:], 1.0)
            ones1 = cp.tile([1, 128], F32)
            nc.vector.memset(ones1[:], 1.0)
            iota4 = cp.tile([128, T], F32)
            nc.gpsimd.iota(
                iota4[:], pattern=[[1, T]], base=0, channel_multiplier=0,
                allow_small_or_imprecise_dtypes=True,
            )
            iota16 = cp.tile([128, 16], F32)
            nc.gpsimd.iota(
                iota16[:], pattern=[[1, 16]], base=0, channel_multiplier=0,
                allow_small_or_imprecise_dtypes=True,
            )
            siota = cp.tile([128, NG], F32)
            nc.gpsimd.iota(
                siota[:], pattern=[[0, BL], [128, 4]], base=0, channel_multiplier=1,
                allow_small_or_imprecise_dtypes=True,
            )

            biasc = cp.tile([128, NG, T], F32)
            nc.vector.tensor_tensor(
                out=biasc[:], in0=fap(pc, 0, [[4, NG], [1, T]]),
                in1=fap(pc, 128, [[0, NG], [1, T]]), op=AL.add,
            )

            # ---------------- gather + projection matmuls ----------------
            lg_ps = pp.tile([128, NG, T], F32)
            if 'nofront' in KV:
                nc.vector.memset(lg_ps[:], 0.25)
            for b in range(BL if 'nofront' not in KV else 0):
                xt = xp.tile([128, 6, S], BF16, tag="xt")
                nc.gpsimd.dma_gather(
                    out_ap=xt[:],
                    in_ap=table_h[:],
                    idxs_ap=gidx[:, b * 32:(b + 1) * 32],
                    num_idxs=S,
                    num_idxs_reg=S,
                    elem_size=D,
                    transpose=True,
                )
                for gl in range(4):
                    for c in range(6):
                        nc.tensor.matmul(
                            lg_ps[:, b * 4 + gl, :],
                            lhsT=xt[:, c, gl * 128:(gl + 1) * 128],
                            rhs=w8b[:, c, b * T:(b + 1) * T],
                            start=(c == 0),
                            stop=(c == 5),
                        )

            # ---------------- softmax epilogue ----------------
            lp = wp.tile([128, NG, T], F32)
            nc.vector.tensor_tensor(out=lp[:], in0=lg_ps[:], in1=biasc[:], op=AL.add)
            mx = wp.tile([128, NG], F32)
            nc.vector.reduce_max(out=mx[:], in_=lp[:], axis=AX.X)
            sh = wp.tile([128, NG, T], F32)
            nc.vector.tensor_tensor(
                out=sh[:], in0=lp[:], in1=fap(mx, 0, [[1, NG], [0, T]]), op=AL.subtract
            )
            eu = wp.tile([128, NG, T], F32)
            nc.scalar.activation(out=eu[:], in_=sh[:], func=AF.Exp)
            sm = wp.tile([128, NG], F32)
            nc.vector.reduce_sum(out=sm[:], in_=eu[:], axis=AX.X)
            rs = wp.tile([128, NG], F32)
            nc.vector.reciprocal(out=rs[:], in_=sm[:])
            qq = wp.tile([128, NG, T], F32)
            nc.vector.tensor_tensor(
                out=qq[:], in0=eu[:], in1=fap(rs, 0, [[1, NG], [0, T]]), op=AL.mult
            )
            lns = wp.tile([128, NG], F32)
            nc.scalar.activation(out=lns[:], in_=sm[:], func=AF.Ln)
            LS = wp.tile([128, NG, T], F32)
            nc.vector.tensor_tensor(
                out=LS[:], in0=sh[:], in1=fap(lns, 0, [[1, NG], [0, T]]), op=AL.subtract
            )

            # q roundtrip through DRAM into scan layout
            nc.sync.dma_start(
                out=dap(qd_h, [[4, 128], [S * T, BL], [512, 4], [1, 4]]),
                in_=qq[:],
            )
            qscan = wp.tile([128, 128], F32)
            nc.sync.dma_start(
                out=qscan[:], in_=dap(qd_h, [[128, 128], [1, 128]])
            )

            # ---------------- gold path ----------------
            mask = wp.tile([128, NG], F32)
            nc.vector.tensor_scalar(
                out=mask[:], in0=words_mm, scalar1=0.0, scalar2=None,
                op0=AL.not_equal,
            )
            msk1 = wp.tile([128, NG], F32)
            nc.vector.tensor_tensor(out=msk1[:], in0=mask[:], in1=sfirst_mm, op=AL.mult)
            oh4 = wp.tile([128, NG, T], F32)
            nc.vector.tensor_tensor(
                out=oh4[:],
                in0=fap(hostc, NG, [[1, NG], [0, T]]),       # target_mm bcast over j
                in1=fap(iota4, 0, [[0, NG], [1, T]]),
                op=AL.is_equal,
            )
            em4 = wp.tile([128, NG, T], F32)
            nc.vector.tensor_tensor(out=em4[:], in0=LS[:], in1=oh4[:], op=AL.mult)
            emit = wp.tile([128, NG], F32)
            nc.vector.reduce_sum(out=emit[:], in_=em4[:], axis=AX.X)
            pair = wp.tile([128, NG], F32)
            nc.vector.tensor_scalar(
                out=pair[:], in0=tprev_mm, scalar1=4.0, scalar2=None, op0=AL.mult
            )
            nc.vector.tensor_tensor(out=pair[:], in0=pair[:], in1=target_mm, op=AL.add)
            oh16 = wp.tile([128, NG, 16], F32)
            nc.vector.tensor_tensor(
                out=oh16[:],
                in0=fap(pair, 0, [[1, NG], [0, 16]]),
                in1=fap(iota16, 0, [[0, NG], [1, 16]]),
                op=AL.is_equal,
            )
            trm = wp.tile([128, NG, 16], F32)
            nc.vector.tensor_tensor(
                out=trm[:], in0=oh16[:],
                in1=fap(pc, 132, [[0, NG], [1, 16]]), op=AL.mult,
            )
            tr = wp.tile([128, NG], F32)
            nc.vector.reduce_sum(out=tr[:], in_=trm[:], axis=AX.X)
            st4 = wp.tile([128, NG, T], F32)
            nc.vector.tensor_tensor(
                out=st4[:], in0=oh4[:], in1=fap(pc, 148, [[0, NG], [1, T]]), op=AL.mult
            )
            st = wp.tile([128, NG], F32)
            nc.vector.reduce_sum(out=st[:], in_=st4[:], axis=AX.X)
            nc.vector.tensor_tensor(out=st[:], in0=st[:], in1=s0sel_mm, op=AL.mult)
            e4 = wp.tile([128, NG, T], F32)
            nc.vector.tensor_tensor(
                out=e4[:], in0=oh4[:], in1=fap(pc, 152, [[0, NG], [1, T]]), op=AL.mult
            )
            eb = wp.tile([128, NG], F32)
            nc.vector.reduce_sum(out=eb[:], in_=e4[:], axis=AX.X)

            cnt_ps = pp2.tile([1, NG], F32)
            nc.tensor.matmul(cnt_ps[:], lhsT=ones128[:], rhs=mask[:], start=True, stop=True)
            cnt8 = wp.tile([1, BL], F32)
            nc.vector.reduce_sum(
                out=cnt8[:], in_=fap(cnt_ps, 0, [[4, BL], [1, 4]]), axis=AX.X
            )
            last8 = wp.tile([1, BL], F32)
            nc.vector.tensor_scalar(
                out=last8[:], in0=cnt8[:], scalar1=-1.0, scalar2=0.0,
                op0=AL.add, op1=AL.max,
            )
            lastrep = wp.tile([1, NG], F32)
            nc.vector.tensor_copy(out=lastrep[:], in_=fap(last8, 0, [[1, BL], [0, 4]]))
            lbc_ps = pp2.tile([128, NG], F32)
            nc.tensor.matmul(lbc_ps[:], lhsT=ones1[:], rhs=lastrep[:], start=True, stop=True)
            ind = wp.tile([128, NG], F32)
            nc.vector.tensor_tensor(out=ind[:], in0=siota[:], in1=lbc_ps[:], op=AL.is_equal)
            etok = wp.tile([128, NG], F32)
            nc.vector.tensor_tensor(out=etok[:], in0=eb[:], in1=ind[:], op=AL.mult)

            tt = wp.tile([128, NG], F32)
            nc.vector.tensor_tensor(out=tt[:], in0=emit[:], in1=mask[:], op=AL.mult)
            t2 = wp.tile([128, NG], F32)
            nc.vector.tensor_tensor(out=t2[:], in0=tr[:], in1=msk1[:], op=AL.mult)
            nc.vector.tensor_tensor(out=tt[:], in0=tt[:], in1=t2[:], op=AL.add)
            nc.vector.tensor_tensor(out=tt[:], in0=tt[:], in1=st[:], op=AL.add)
            nc.vector.tensor_tensor(out=tt[:], in0=tt[:], in1=etok[:], op=AL.add)
            gold_ps = pp2.tile([1, NG], F32)
            nc.tensor.matmul(gold_ps[:], lhsT=ones128[:], rhs=tt[:], start=True, stop=True)
            gold8 = wp.tile([1, BL], F32)
            nc.vector.reduce_sum(
                out=gold8[:], in_=fap(gold_ps, 0, [[4, BL], [1, 4]]), axis=AX.X
            )
            gold_p = wp.tile([BL, 1], F32)
            nc.sync.dma_start(out=gold_p[:], in_=gold8[:])

            # ---------------- CRF scan ----------------
            # Precompute all step matrices:
            #   Mf[cl, tau, k, j] = mb * expT[k, j] * q[cl, tau, j] + (1 - mb) * I
            mb = wp.tile([128, NG], F32)
            nc.vector.tensor_scalar(
                out=mb[:], in0=words_sc, scalar1=0.0, scalar2=None, op0=AL.not_equal
            )
            nc.vector.tensor_tensor(out=mb[:], in0=mb[:], in1=sfirst_sc, op=AL.mult)
            inv = wp.tile([128, NG], F32)
            nc.vector.tensor_scalar(
                out=inv[:], in0=mb[:], scalar1=-1.0, scalar2=1.0,
                op0=AL.mult, op1=AL.add,
            )
            Mf = wp.tile([128, NG, 16], F32)
            nc.vector.tensor_tensor(
                out=fap(Mf, 0, [[16, NG], [4, 4], [1, 4]]),
                in0=fap(qscan, 0, [[4, NG], [0, 4], [1, 4]]),
                in1=fap(expT_bc, 0, [[0, NG], [4, 4], [1, 4]]),
                op=AL.mult,
            )
            nc.vector.tensor_tensor(
                out=fap(Mf, 0, [[16, NG], [1, 16]]),
                in0=fap(Mf, 0, [[16, NG], [1, 16]]),
                in1=fap(mb, 0, [[1, NG], [0, 16]]),
                op=AL.mult,
            )
            nc.vector.tensor_tensor(
                out=fap(Mf, 0, [[16, NG], [5, 4]]),
                in0=fap(Mf, 0, [[16, NG], [5, 4]]),
                in1=fap(inv, 0, [[1, NG], [0, 4]]),
                op=AL.add,
            )

            # pairwise tree over the 8 step matrices of each chunk:
            # level A: 4 products per c_lo, level B: 2, level C: 1 -> Pst
            tmpA = wp.tile([128, 1024], F32)
            A2 = wp.tile([128, 16, 16], F32)     # (c_lo, pair) x 16
            nc.vector.tensor_tensor(
                out=fap(tmpA, 0, [[16, 64], [4, 4], [1, 4]]),
                in0=fap(Mf, 0, [[32, 16], [1, 16], [0, 4]]),
                in1=fap(Mf, 16, [[32, 16], [0, 4], [1, 16]]),
                op=AL.mult,
            )
            nc.vector.reduce_sum(
                out=fap(A2, 0, [[4, 64], [1, 4]]),
                in_=fap(tmpA, 0, [[16, 64], [1, 4], [4, 4]]),
                axis=AX.X,
            )
            B2 = wp.tile([128, 8, 16], F32)      # (c_lo, bp) x 16
            nc.vector.tensor_tensor(
                out=fap(tmpA, 0, [[16, 32], [4, 4], [1, 4]]),
                in0=fap(A2, 0, [[32, 8], [1, 16], [0, 4]]),
                in1=fap(A2, 16, [[32, 8], [0, 4], [1, 16]]),
                op=AL.mult,
            )
            nc.vector.reduce_sum(
                out=fap(B2, 0, [[4, 32], [1, 4]]),
                in_=fap(tmpA, 0, [[16, 32], [1, 4], [4, 4]]),
                axis=AX.X,
            )
            Pst = wp.tile([128, 4, 16], F32)
            nc.vector.tensor_tensor(
                out=fap(tmpA, 0, [[16, 16], [4, 4], [1, 4]]),
                in0=fap(B2, 0, [[32, 4], [1, 16], [0, 4]]),
                in1=fap(B2, 16, [[32, 4], [0, 4], [1, 16]]),
                op=AL.mult,
            )
            nc.vector.reduce_sum(
                out=fap(Pst, 0, [[4, 16], [1, 4]]),
                in_=fap(tmpA, 0, [[16, 16], [1, 4], [4, 4]]),
                axis=AX.X,
            )
            # combine level 1: pairs over c_lo -> P2 [128, 2, 16]
            # iter (pair, i, k, j); tmp2 layout [pair][i][k][j]
            tmp2 = wp.tile([128, 128], F32)
            nc.vector.tensor_tensor(
                out=fap(tmp2, 0, [[16, 8], [4, 4], [1, 4]]),
                in0=fap(Pst, 0, [[32, 2], [1, 16], [0, 4]]),
                in1=fap(Pst, 16, [[32, 2], [0, 4], [1, 16]]),
                op=AL.mult,
            )
            P2 = wp.tile([128, 2, 16], F32)
            nc.vector.reduce_sum(
                out=fap(P2, 0, [[4, 8], [1, 4]]),
                in_=fap(tmp2, 0, [[16, 8], [1, 4], [4, 4]]),
                axis=AX.X,
            )
            # combine level 2 -> P4z[:, 0:16], lacc -> P4z[:, 16], q0 -> P4z[:, 17:21]
            P4z = wp.tile([128, 21], F32)
            tmp3 = wp.tile([128, 64], F32)
            # iter (i, k, j); tmp3 layout [i][k][j]
            nc.vector.tensor_tensor(
                out=fap(tmp3, 0, [[16, 4], [4, 4], [1, 4]]),
                in0=fap(P2, 0, [[1, 16], [0, 4]]),
                in1=fap(P2, 16, [[0, 4], [1, 16]]),
                op=AL.mult,
            )
            nc.vector.reduce_sum(
                out=fap(P4z, 0, [[4, 4], [1, 4]]),
                in_=fap(tmp3, 0, [[16, 4], [1, 4], [4, 4]]),
                axis=AX.X,
            )
            m1 = wp.tile([128, 1], F32)
            nc.vector.reduce_max(out=m1[:], in_=fap(P4z, 0, [[1, 16]]), axis=AX.X)
            rm1 = wp.tile([128, 1], F32)
            nc.vector.reciprocal(out=rm1[:], in_=m1[:])
            nc.vector.tensor_tensor(
                out=fap(P4z, 0, [[1, 16]]), in0=fap(P4z, 0, [[1, 16]]),
                in1=fap(rm1, 0, [[0, 16]]), op=AL.mult,
            )
            nc.scalar.activation(
                out=fap(P4z, 16, [[1, 1]]), in_=m1[:], func=AF.Ln
            )
            nc.vector.tensor_copy(out=fap(P4z, 17, [[1, 4]]), in_=qscan[:, 0:4])

            # fold partitions via DRAM roundtrip: [128, 21] -> [8, 16*21]
            Pz = wp.tile([BL, 16 * 21], F32)
            nc.sync.dma_start(out=Pz[:], in_=P4z[:])

            # combine levels 3-6 (free dim), no per-level rescale
            # each: iter (pair, i, k, j); t3 layout [pair][i][k][j]
            t3 = wp.tile([BL, 512], F32)
            P3 = wp.tile([BL, 8, 16], F32)
            nc.vector.tensor_tensor(
                out=fap(t3, 0, [[16, 32], [4, 4], [1, 4]]),
                in0=fap(Pz, 0, [[42, 8], [1, 16], [0, 4]]),
                in1=fap(Pz, 21, [[42, 8], [0, 4], [1, 16]]),
                op=AL.mult,
            )
            nc.vector.reduce_sum(
                out=fap(P3, 0, [[4, 32], [1, 4]]),
                in_=fap(t3, 0, [[16, 32], [1, 4], [4, 4]]),
                axis=AX.X,
            )
            P4b = wp.tile([BL, 4, 16], F32)
            nc.vector.tensor_tensor(
                out=fap(t3, 0, [[16, 16], [4, 4], [1, 4]]),
                in0=fap(P3, 0, [[32, 4], [1, 16], [0, 4]]),
                in1=fap(P3, 16, [[32, 4], [0, 4], [1, 16]]),
                op=AL.mult,
            )
            nc.vector.reduce_sum(
                out=fap(P4b, 0, [[4, 16], [1, 4]]),
                in_=fap(t3, 0, [[16, 16], [1, 4], [4, 4]]),
                axis=AX.X,
            )
            P5 = wp.tile([BL, 2, 16], F32)
            nc.vector.tensor_tensor(
                out=fap(t3, 0, [[16, 8], [4, 4], [1, 4]]),
                in0=fap(P4b, 0, [[32, 2], [1, 16], [0, 4]]),
                in1=fap(P4b, 16, [[32, 2], [0, 4], [1, 16]]),
                op=AL.mult,
            )
            nc.vector.reduce_sum(
                out=fap(P5, 0, [[4, 8], [1, 4]]),
                in_=fap(t3, 0, [[16, 8], [1, 4], [4, 4]]),
                axis=AX.X,
            )
            Ptot = wp.tile([BL, 16], F32)
            nc.vector.tensor_tensor(
                out=fap(t3, 0, [[16, 4], [4, 4], [1, 4]]),
                in0=fap(P5, 0, [[1, 16], [0, 4]]),
                in1=fap(P5, 16, [[0, 4], [1, 16]]),
                op=AL.mult,
            )
            nc.vector.reduce_sum(
                out=fap(Ptot, 0, [[4, 4], [1, 4]]),
                in_=fap(t3, 0, [[16, 4], [1, 4], [4, 4]]),
                axis=AX.X,
            )
            laccs = wp.tile([BL, 1], F32)
            nc.vector.reduce_sum(out=laccs[:], in_=fap(Pz, 16, [[21, 16]]), axis=AX.X)

            # final: alpha = (q0 * expS) @ Ptot ; Z = sum(alpha * expE)
            q0s = wp.tile([BL, T], F32)
            nc.vector.tensor_tensor(
                out=q0s[:], in0=fap(Pz, 17, [[1, 4]]), in1=expS[0:BL, :], op=AL.mult
            )
            ta = wp.tile([BL, 16], F32)
            nc.vector.tensor_tensor(
                out=fap(ta, 0, [[4, 4], [1, 4]]),
                in0=fap(q0s, 0, [[0, 4], [1, 4]]),
                in1=fap(Ptot, 0, [[1, 4], [4, 4]]),
                op=AL.mult,
            )
            av = wp.tile([BL, T], F32)
            nc.vector.reduce_sum(out=av[:], in_=fap(ta, 0, [[4, 4], [1, 4]]), axis=AX.X)
            ze = wp.tile([BL, T], F32)
            nc.vector.tensor_tensor(out=ze[:], in0=av[:], in1=expE[0:BL, :], op=AL.mult)
            Z = wp.tile([BL, 1], F32)
            nc.vector.reduce_sum(out=Z[:], in_=ze[:], axis=AX.X)
            lnZ = wp.tile([BL, 1], F32)
            nc.scalar.activation(out=lnZ[:], in_=Z[:], func=AF.Ln)
            norm = wp.tile([BL, 1], F32)
            nc.vector.tensor_tensor(out=norm[:], in0=lnZ[:], in1=laccs[:], op=AL.add)
            nllp = wp.tile([BL, 1], F32)
            nc.vector.tensor_tensor(out=nllp[:], in0=norm[:], in1=gold_p[:], op=AL.subtract)
            nc.sync.dma_start(out=nll_h[:], in_=nllp[:])

    nc.compile()
    _CACHE["nc"] = nc
    return nc


def _prep_core(words, target, corpus, shared_b, domain_A, domain_b,
               trans_m, start_scores, end_scores):
    w = np.asarray(words, np.int64)
    t = np.asarray(target, np.int64)

    def mm(a):
        return np.ascontiguousarray(
            np.asarray(a, np.float64).reshape(BL, 4, 128)
            .transpose(2, 0, 1).reshape(128, NG)
        ).astype(np.float32)

    def sc(a):
        return np.ascontiguousarray(
            np.asarray(a, np.float64).reshape(BL, 16, 4, 8).reshape(128, 32)
        ).astype(np.float32)

    sfirst = np.ones((BL, S)); sfirst[:, 0] = 0.0
    tprev = np.concatenate([np.zeros((BL, 1), np.int64), t[:, :-1]], axis=1)
    hostf = np.stack([
        mm(w), mm(t), mm(tprev), mm(sfirst), mm(1.0 - sfirst), sc(w), sc(sfirst),
    ]).astype(np.float32)

    gidx = np.zeros((128, NG * 8), np.int16)
    for b in range(BL):
        gidx[:16, b * 32:(b + 1) * 32] = w[b].reshape(32, 16).T.astype(np.int16)

    hostp = np.zeros(176, np.float32)
    hostp[0:128] = np.repeat(
        np.asarray(domain_b, np.float32)[corpus][:, None, :], 4, axis=1
    ).reshape(-1)
    hostp[128:132] = np.asarray(shared_b, np.float32)
    hostp[132:148] = np.asarray(trans_m, np.float32).reshape(-1)
    hostp[148:152] = np.asarray(start_scores, np.float32)
    hostp[152:156] = np.asarray(end_scores, np.float32)

    a8t = np.ascontiguousarray(
        np.asarray(domain_A, np.float32)[corpus].transpose(1, 0, 2).reshape(D, BL * T)
    )
    return hostf, gidx, hostp, a8t


def kernel(_trace=False, **inputs):
    from concourse.bass_utils import run_bass_kernel_spmd

    words = np.asarray(inputs["words"])
    target = np.asarray(inputs["target"])
    corpus = np.asarray(inputs["corpus"])
    table_bf16 = np.ascontiguousarray(
        np.asarray(inputs["embed_table"], np.float32).astype(ml_dtypes.bfloat16)
    )
    sw = np.ascontiguousarray(np.asarray(inputs["shared_W"], np.float32))

    nc = _build()
    in_maps = []
    for k in range(NCORES):
        sl = slice(k * BL, (k + 1) * BL)
        hostf, gidx, hostp, a8t = _prep_core(
            words[sl], target[sl], corpus[sl], inputs["shared_b"],
            inputs["domain_A"], inputs["domain_b"], inputs["trans_m"],
            inputs["start_scores"], inputs["end_scores"],
        )
        in_maps.append({
            "table": table_bf16, "gidx": gidx, "hostf": hostf,
            "hostp": hostp, "a8t": a8t, "sw": sw,
        })
    res = run_bass_kernel_spmd(
        nc, in_maps, core_ids=list(range(NCORES)), trace=_trace,
    )
    if _trace:
        print("exec_time_ns:", res.exec_time_ns,
              "mean:", res.mean_exec_time_ns,
              "trace:", (res.instructions_and_trace or (None, None))[1])
    out = np.concatenate([res.results[k]["nll"] for k in range(NCORES)])
    return out.astype(np.float32)



# revision 15
# speedup vs baseline: 1.2444x; 1.2444x over previous
"""Trainium2 Bass kernel for nn_Bert_Proj_CRF (BERT projection + CRF NLL).

Strategy (data-parallel over batch, 8 NeuronCores x 8 sequences):
  - Embedding rows are gathered in fp8 (e3m4, x64 scale) with a transpose
    gather: byte d of a row lands at partition (d//2)%128, free offset
    (d//512, token, d%2).  Weight rows are host-permuted to match, so the
    projection matmul runs directly on the gathered layout (fp8 PE).
  - No softmax: NLL = ln(Z_scan(exp(raw))) - sum(raw[tgt]) + host consts,
    because the per-token log-sum-exp terms cancel between the CRF
    normalizer and the gold score (up to a tiny mask[0] correction).
  - CRF normalizer via a pairwise product tree of 4x4 matrices
    M_t = (exp(trans)*exp(bias_b)/4) * u_t (identity where masked), with
    the /4 keeping magnitudes O(1) (host adds n_unmask*ln4 back).
    Tokens are gathered in bit-reversed lane order so the tree pairs
    partition halves at every level - no transpose DMA round trips.
"""

import numpy as np
import ml_dtypes

import concourse.bass as bass
import concourse.bacc as bacc
import concourse.tile as tile
import concourse.mybir as mybir

V, D, T = 21128, 768, 4
B, S = 64, 512
NCORES = 8
BL = B // NCORES            # 8 sequences per core
NGATH = 8                   # gathers per core (1 sequence each)
TPG = S                     # 512 tokens per gather
SC = 64.0                   # fp8 quantization scale
ISC = 1.0 / (SC * SC)
F32 = mybir.dt.float32
BF16 = mybir.dt.bfloat16
F8 = mybir.dt.float8e3
I16 = mybir.dt.int16
AF = mybir.ActivationFunctionType
AL = mybir.AluOpType
AX = mybir.AxisListType

_REV7 = np.array([int(format(p, "07b")[::-1], 2) for p in range(128)])


def fap(t, off, dims):
    """AP over tile t's partition dim with custom free dims (element units)."""
    base = t if isinstance(t, bass.AP) else t[:]
    return bass.AP(
        tensor=base.tensor,
        offset=base.offset + off,
        ap=[list(base.ap[0])] + [list(d) for d in dims],
    )


def pap(t, p0, p1, off, dims):
    """Like fap but restricted to partitions [p0, p1)."""
    base = t if isinstance(t, bass.AP) else t[:]
    pd = list(base.ap[0])
    return bass.AP(
        tensor=base.tensor,
        offset=base.offset + p0 * pd[0] + off,
        ap=[[pd[0], p1 - p0]] + [list(d) for d in dims],
    )


def dap(handle, ap):
    return bass.AP(tensor=handle, offset=0, ap=[list(d) for d in ap])


_CACHE = {}


def _build():
    if "nc" in _CACHE:
        return _CACHE["nc"]
    nc = bacc.Bacc()

    # fp8 table bytes typed as bf16 (D/2 elems): the 16-bit transpose gather
    # moves byte pairs; matmuls bitcast the gathered tile back to fp8.
    table_h = nc.dram_tensor("table", [V, D // 2], BF16, kind="ExternalInput")
    gidx_h = nc.dram_tensor("gidx", [128, NGATH * TPG // 16], I16, kind="ExternalInput")
    w8_h = nc.dram_tensor("w8", [128, 6 * BL * T], F8, kind="ExternalInput")
    e4x_h = nc.dram_tensor("e4x", [BL * 4 * 16], BF16, kind="ExternalInput")
    m4_h = nc.dram_tensor("m4", [128, BL * 4], BF16, kind="ExternalInput")
    dinv_h = nc.dram_tensor("dinv", [128, BL * 4], BF16, kind="ExternalInput")
    ohm_h = nc.dram_tensor("ohm", [128, BL * 16], BF16, kind="ExternalInput")
    ident_h = nc.dram_tensor("ident", [128, 128], BF16, kind="ExternalInput")
    tailc_h = nc.dram_tensor("tailc", [128], F32, kind="ExternalInput")
    nll_h = nc.dram_tensor("nll", [BL], F32, kind="ExternalOutput")

    with tile.TileContext(nc) as tc:
        with (
            tc.tile_pool(name="consts", bufs=1) as cp,
            tc.tile_pool(name="xt", bufs=NGATH) as xp,
            tc.tile_pool(name="work", bufs=1) as wp,
            tc.tile_pool(name="psum", bufs=1, space="PSUM") as pp,
            tc.tile_pool(name="psum2", bufs=1, space="PSUM") as pp2,
        ):
            # ---- inputs: gidx FIRST so gathers start asap ----
            gidx = cp.tile([128, NGATH * TPG // 16], I16)
            nc.sync.dma_start(out=gidx[:], in_=gidx_h[:])
            w8 = cp.tile([128, 6 * BL * T], F8)
            nc.sync.dma_start(out=w8[:], in_=w8_h[:])
            e4x = cp.tile([128, BL * 4 * 16], BF16)
            nc.sync.dma_start(out=e4x[:], in_=dap(e4x_h, [[0, 128], [1, BL * 4 * 16]]))
            m4 = cp.tile([128, BL * 4], BF16)
            nc.sync.dma_start(out=m4[:], in_=m4_h[:])
            dinv = cp.tile([128, BL * 4], BF16)
            nc.sync.dma_start(out=dinv[:], in_=dinv_h[:])
            ohm = cp.tile([128, BL * 16], BF16)
            nc.sync.dma_start(out=ohm[:], in_=ohm_h[:])
            tailc = cp.tile([1, 128], F32)
            nc.sync.dma_start(out=tailc[:], in_=dap(tailc_h, [[0, 1], [1, 128]]))
            ident = cp.tile([128, 128], BF16)
            nc.sync.dma_start(out=ident[:], in_=ident_h[:])
            ones128 = cp.tile([128, 1], F32)
            nc.vector.memset(ones128[:], 1.0)

            lg = pp.tile([128, BL * 4, T], F32)      # raw logits * 4096
            u = wp.tile([128, BL * 4 * T], BF16)     # exp(raw)
            v = wp.tile([128, BL * 4 * T], BF16)     # u * mask
            em = wp.tile([128, BL * 16], F32)        # raw*onehot*mask/4096
            Mf = wp.tile([128, BL * 4 * 16], BF16)   # step matrices
            t1 = wp.tile([128, 512], BF16)
            t2 = wp.tile([128, 256], BF16)
            P1t = wp.tile([128, 256], BF16)          # L1 fold out (b,pr,16)
            B4a = wp.tile([128, BL * 16], BF16)      # 4-step blocks
            B4b = wp.tile([128, BL * 16], BF16)

            xts = []
            for g in range(NGATH):
                xt = xp.tile([128, 3, TPG], BF16, tag="xt")
                xts.append(xt)
                nc.gpsimd.dma_gather(
                    out_ap=xt[:],
                    in_ap=table_h[:],
                    idxs_ap=gidx[:, g * (TPG // 16):(g + 1) * (TPG // 16)],
                    num_idxs=TPG,
                    num_idxs_reg=TPG,
                    elem_size=D // 2,
                    transpose=True,
                )

            for b in range(BL):
                # ---- projection matmuls (fp8 view): lg[:, b*4+gl, :] ----
                xf8 = xts[b][:].bitcast(F8)
                for gl in range(4):
                    tok0 = gl * 128
                    for cb in range(6):
                        c16, bit = cb // 2, cb % 2
                        lhsT = fap(xf8, c16 * 2 * TPG + tok0 * 2 + bit, [[2, 128]])
                        nc.tensor.matmul(
                            lg[:, b * 4 + gl, :],
                            lhsT=lhsT,
                            rhs=w8[:, cb * BL * T + b * T:(cb * BL * T + b * T) + T],
                            start=(cb == 0),
                            stop=(cb == 5),
                        )

            for g in range(4):
                # ---- epilogue per pair of sequences (cols 32g..) ----
                c0, c1 = 32 * g, 32 * (g + 1)
                lg_sl = fap(lg, c0, [[1, 32]])
                nc.scalar.activation(out=fap(u, c0, [[1, 32]]), in_=lg_sl,
                                     func=AF.Exp, scale=ISC)
                nc.vector.tensor_tensor(
                    out=fap(em, c0, [[1, 32]]), in0=lg_sl,
                    in1=fap(ohm, c0, [[1, 32]]), op=AL.mult,
                )
                nc.vector.tensor_tensor(
                    out=fap(v, c0, [[1, 32]]), in0=fap(u, c0, [[1, 32]]),
                    in1=fap(m4, 8 * g, [[1, 8], [0, 4]]), op=AL.mult,
                )
                # Mf[:, (b,gl), k, j] = v[j] * E4x[(b,gl),k,j];  diag += dinv
                nc.vector.tensor_tensor(
                    out=fap(Mf, 128 * g, [[1, 128]]),
                    in0=fap(v, c0, [[4, 8], [0, 4], [1, 4]]),
                    in1=fap(e4x, 128 * g, [[1, 128]]),
                    op=AL.mult,
                )
                nc.vector.tensor_tensor(
                    out=fap(Mf, 128 * g, [[16, 8], [5, 4]]),
                    in0=fap(Mf, 128 * g, [[16, 8], [5, 4]]),
                    in1=fap(dinv, 8 * g, [[1, 8], [0, 4]]),
                    op=AL.add,
                )
                # ---- within-lane fold L1: (gl0*gl1), (gl2*gl3) ----
                # t1[(bp,pr), i, k, j] = A[i,k]*B[k,j]
                nc.vector.tensor_tensor(
                    out=fap(t1, 0, [[64, 4], [1, 64]]),
                    in0=fap(Mf, 128 * g, [[32, 4], [1, 16], [0, 4]]),
                    in1=fap(Mf, 128 * g + 16, [[32, 4], [0, 4], [1, 16]]),
                    op=AL.mult,
                )
                # k-sum: t2[(bp,pr), i, k2, j]; then P1t[(bp,pr), 16]
                nc.vector.tensor_tensor(
                    out=fap(t2, 0, [[32, 4], [8, 4], [1, 4]]),
                    in0=fap(t1, 0, [[64, 4], [16, 4], [1, 4]]),
                    in1=fap(t1, 4, [[64, 4], [16, 4], [1, 4]]),
                    op=AL.add,
                )
                nc.vector.tensor_tensor(
                    out=fap(t2, 4, [[32, 4], [8, 4], [1, 4]]),
                    in0=fap(t1, 8, [[64, 4], [16, 4], [1, 4]]),
                    in1=fap(t1, 12, [[64, 4], [16, 4], [1, 4]]),
                    op=AL.add,
                )
                nc.vector.tensor_tensor(
                    out=fap(P1t, 64 * g, [[16, 4], [1, 16]]),
                    in0=fap(t2, 0, [[32, 4], [8, 4], [1, 4]]),
                    in1=fap(t2, 4, [[32, 4], [8, 4], [1, 4]]),
                    op=AL.add,
                )
                # ---- L2: per bp, pair products -> B4[:, b, 16] ----
                nc.vector.tensor_tensor(
                    out=fap(t1, 0, [[64, 2], [1, 64]]),
                    in0=fap(P1t, 64 * g, [[32, 2], [1, 16], [0, 4]]),
                    in1=fap(P1t, 64 * g + 16, [[32, 2], [0, 4], [1, 16]]),
                    op=AL.mult,
                )
                nc.vector.tensor_tensor(
                    out=fap(t2, 0, [[32, 2], [8, 4], [1, 4]]),
                    in0=fap(t1, 0, [[64, 2], [16, 4], [1, 4]]),
                    in1=fap(t1, 4, [[64, 2], [16, 4], [1, 4]]),
                    op=AL.add,
                )
                nc.vector.tensor_tensor(
                    out=fap(t2, 4, [[32, 2], [8, 4], [1, 4]]),
                    in0=fap(t1, 8, [[64, 2], [16, 4], [1, 4]]),
                    in1=fap(t1, 12, [[64, 2], [16, 4], [1, 4]]),
                    op=AL.add,
                )
                nc.vector.tensor_tensor(
                    out=fap(B4a, 32 * g, [[16, 2], [1, 16]]),
                    in0=fap(t2, 0, [[32, 2], [8, 4], [1, 4]]),
                    in1=fap(t2, 4, [[32, 2], [8, 4], [1, 4]]),
                    op=AL.add,
                )

            # ---- cross-partition tree: 7 levels of halves pairing ----
            # DVE can't read two SBUF operands at different base partitions,
            # so the upper half is first shifted down via a PE matmul with a
            # sliced identity (PSUM operand dodges the equal-base rule).
            psh = pp2.tile([128, BL * 16], F32)
            cur, nxt = B4a, B4b
            for lvl in range(7):
                half = 64 >> lvl
                nc.tensor.matmul(
                    pap(psh, 0, half, 0, [[1, BL * 16]]),
                    lhsT=pap(ident, 0, 2 * half, half, [[1, half]]),
                    rhs=pap(cur, 0, 2 * half, 0, [[1, BL * 16]]),
                    start=True, stop=True,
                )
                nc.vector.tensor_tensor(
                    out=pap(t1, 0, half, 0, [[64, 8], [1, 64]]),
                    in0=pap(cur, 0, half, 0, [[16, 8], [1, 16], [0, 4]]),
                    in1=pap(psh, 0, half, 0, [[16, 8], [0, 4], [1, 16]]),
                    op=AL.mult,
                )
                nc.vector.tensor_tensor(
                    out=pap(t2, 0, half, 0, [[32, 8], [8, 4], [1, 4]]),
                    in0=pap(t1, 0, half, 0, [[64, 8], [16, 4], [1, 4]]),
                    in1=pap(t1, 0, half, 4, [[64, 8], [16, 4], [1, 4]]),
                    op=AL.add,
                )
                nc.vector.tensor_tensor(
                    out=pap(t2, 0, half, 4, [[32, 8], [8, 4], [1, 4]]),
                    in0=pap(t1, 0, half, 8, [[64, 8], [16, 4], [1, 4]]),
                    in1=pap(t1, 0, half, 12, [[64, 8], [16, 4], [1, 4]]),
                    op=AL.add,
                )
                nc.vector.tensor_tensor(
                    out=pap(nxt, 0, half, 0, [[16, 8], [1, 16]]),
                    in0=pap(t2, 0, half, 0, [[32, 8], [8, 4], [1, 4]]),
                    in1=pap(t2, 0, half, 4, [[32, 8], [8, 4], [1, 4]]),
                    op=AL.add,
                )
                cur, nxt = nxt, cur

            # ---- tail on partition 0: Z, gold, corrections ----
            sv = fap(tailc, 0, [[1, 32]])       # exp(bias+start) per (b,j)
            ee = fap(tailc, 32, [[1, 32]])      # exp(end) per (b,j)
            eb0 = fap(tailc, 64, [[1, 32]])     # exp(bias) per (b,j)
            hg = fap(tailc, 96, [[1, 8]])       # n_unmask*ln4 - host_gold
            cm0 = fap(tailc, 104, [[1, 8]])     # mask0 - 1

            a0 = wp.tile([1, 32], F32)          # u0 * exp(bias+start)
            nc.vector.tensor_tensor(
                out=a0[:], in0=pap(u, 0, 1, 0, [[16, 8], [1, 4]]), in1=sv,
                op=AL.mult,
            )
            tz = wp.tile([1, 128], F32)
            nc.vector.tensor_tensor(
                out=tz[:], in0=pap(cur, 0, 1, 0, [[1, 128]]),
                in1=fap(a0, 0, [[4, 8], [1, 4], [0, 4]]),
                op=AL.mult,
            )
            tz2 = wp.tile([1, 64], F32)
            nc.vector.tensor_tensor(
                out=tz2[:], in0=fap(tz, 0, [[16, 8], [8, 2], [1, 4]]),
                in1=fap(tz, 4, [[16, 8], [8, 2], [1, 4]]), op=AL.add,
            )
            za = wp.tile([1, 32], F32)
            nc.vector.tensor_tensor(
                out=za[:], in0=fap(tz2, 0, [[8, 8], [1, 4]]),
                in1=fap(tz2, 4, [[8, 8], [1, 4]]), op=AL.add,
            )
            ze = wp.tile([1, 32], F32)
            nc.vector.tensor_tensor(out=ze[:], in0=za[:], in1=ee, op=AL.mult)
            z8 = wp.tile([1, BL], F32)
            nc.vector.reduce_sum(out=z8[:], in_=fap(ze, 0, [[4, 8], [1, 4]]), axis=AX.X)
            lnz = wp.tile([1, BL], F32)
            nc.scalar.activation(out=lnz[:], in_=z8[:], func=AF.Ln)

            # mask0 correction: (m0-1)*ln(sum_j u0*exp(bias))
            a0e = wp.tile([1, 32], F32)
            nc.vector.tensor_tensor(
                out=a0e[:], in0=pap(u, 0, 1, 0, [[16, 8], [1, 4]]), in1=eb0,
                op=AL.mult,
            )
            s0 = wp.tile([1, BL], F32)
            nc.vector.reduce_sum(out=s0[:], in_=fap(a0e, 0, [[4, 8], [1, 4]]), axis=AX.X)
            lns0 = wp.tile([1, BL], F32)
            nc.scalar.activation(out=lns0[:], in_=s0[:], func=AF.Ln)
            corr = wp.tile([1, BL], F32)
            nc.vector.tensor_tensor(out=corr[:], in0=lns0[:], in1=cm0, op=AL.mult)

            # gold emit: ones^T @ em -> per-column sums -> per-b
            gold_ps = pp2.tile([1, BL * 16], F32)
            nc.tensor.matmul(gold_ps[:], lhsT=ones128[:], rhs=em[:], start=True, stop=True)
            emit8 = wp.tile([1, BL], F32)
            nc.vector.reduce_sum(
                out=emit8[:], in_=fap(gold_ps, 0, [[16, 8], [1, 16]]), axis=AX.X
            )

            nll = wp.tile([1, BL], F32)
            nc.vector.tensor_tensor(out=nll[:], in0=lnz[:], in1=hg, op=AL.add)
            nc.vector.tensor_tensor(out=nll[:], in0=nll[:], in1=emit8[:], op=AL.subtract)
            nc.vector.tensor_tensor(out=nll[:], in0=nll[:], in1=corr[:], op=AL.add)
            nc.sync.dma_start(out=nll_h[:], in_=nll[:])

    nc.compile()
    _CACHE["nc"] = nc
    return nc


def _prep_core(words, target, corpus, shared_W, shared_b, domain_A, domain_b,
               trans_m, start_scores, end_scores):
    w = np.asarray(words, np.int64)
    t = np.asarray(target, np.int64)
    sw = np.asarray(shared_W, np.float32)
    sb = np.asarray(shared_b, np.float32)
    dA = np.asarray(domain_A, np.float32)
    db = np.asarray(domain_b, np.float32)
    tm = np.asarray(trans_m, np.float32)
    ss = np.asarray(start_scores, np.float32)
    es = np.asarray(end_scores, np.float32)

    rev = _REV7
    # token s for (b, gl, p): s = 4*rev7(p) + gl
    s_of = (4 * rev[None, :] + np.arange(4)[:, None, None]).reshape(1, 4, 128)  # (1,gl,p)

    # gidx: gather g = sequence g, flat order (gl, p).  For 768B rows the
    # engine reads the 16-wide index wrap from partitions 16..31 (measured).
    gidx = np.zeros((128, NGATH * TPG // 16), np.int16)
    for g in range(NGATH):
        seq = np.empty(TPG, np.int64)
        for gl in range(4):
            seq[gl * 128:gl * 128 + 128] = w[g][4 * rev + gl]
        gidx[16:32, g * (TPG // 16):(g + 1) * (TPG // 16)] = (
            seq.reshape(TPG // 16, 16).T.astype(np.int16))

    W = sw[None] + dA[corpus]                      # (BL, D, T)
    bias = sb[None] + db[corpus]                   # (BL, T)
    W8q = np.asarray((W * SC).astype(ml_dtypes.float8_e3m4))
    # w8[p, cb, b, j] = W8q[b, 2*((cb//2)*128+p) + cb%2, j]
    cb = np.arange(6)
    p = np.arange(128)
    drow = 2 * ((cb[None, :] // 2) * 128 + p[:, None]) + (cb[None, :] % 2)  # (128, 6)
    w8 = np.ascontiguousarray(
        W8q[:, drow, :].transpose(1, 2, 0, 3).reshape(128, 6 * BL * T))

    eT = np.exp(tm)                                # (4,4) k,j
    e4 = (eT[None, :, :] * np.exp(bias)[:, None, :] / 4.0)   # (BL, k, j)
    e4x = np.ascontiguousarray(
        np.broadcast_to(e4[:, None, :, :], (BL, 4, 4, 4)).reshape(-1)
    ).astype(ml_dtypes.bfloat16)

    mask = (w != 0)                                # (BL, S)
    m = mask.astype(np.float32)
    # scan mask per (p, b, gl): step s = 4*rev(p)+gl, zero at s==0
    sm = np.zeros((128, BL, 4), np.float32)
    for gl in range(4):
        s_idx = 4 * rev + gl                       # (128,)
        sm[:, :, gl] = m[:, s_idx].T
        if gl == 0:
            sm[rev == 0, :, 0] = 0.0               # s==0 -> identity
    m4 = np.ascontiguousarray(sm.reshape(128, BL * 4)).astype(ml_dtypes.bfloat16)
    dinv = np.ascontiguousarray(1.0 - sm.reshape(128, BL * 4)).astype(ml_dtypes.bfloat16)

    # ohm[p, b, gl, j] = (target==j)*(mask)*ISC at s = 4*rev(p)+gl
    ohm = np.zeros((128, BL, 4, 4), np.float32)
    for gl in range(4):
        s_idx = 4 * rev + gl
        oh = (np.eye(4, dtype=np.float32)[t[:, s_idx]] * m[:, s_idx, None])  # (BL,128,4)
        ohm[:, :, gl, :] = oh.transpose(1, 0, 2)
    ohm = np.ascontiguousarray((ohm * ISC).reshape(128, BL * 16)).astype(ml_dtypes.bfloat16)

    bidx = np.arange(BL)
    tr = tm[t[:, :-1], t[:, 1:]] * m[:, 1:]
    last_idx = np.maximum(m.sum(1).astype(np.int64) - 1, 0)
    host_gold = ((bias[bidx[:, None], t] * m).sum(1) + tr.sum(1)
                 + ss[t[:, 0]] + es[t[bidx, last_idx]])
    n_unmask = m[:, 1:].sum(1)

    tailc = np.zeros(128, np.float32)
    tailc[0:32] = np.exp(bias + ss[None, :]).reshape(-1)
    tailc[32:64] = np.tile(np.exp(es), BL)
    tailc[64:96] = np.exp(bias).reshape(-1)
    tailc[96:104] = n_unmask * np.log(4.0) - host_gold
    tailc[104:112] = m[:, 0] - 1.0
    ident = np.eye(128, dtype=ml_dtypes.bfloat16)
    return gidx, w8, e4x, m4, dinv, ohm, tailc, ident


def kernel(_trace=False, **inputs):
    from concourse.bass_utils import run_bass_kernel_spmd

    words = np.asarray(inputs["words"])
    target = np.asarray(inputs["target"])
    corpus = np.asarray(inputs["corpus"])
    table8 = np.asarray(
        (np.asarray(inputs["embed_table"], np.float32) * SC).astype(ml_dtypes.float8_e3m4)
    ).view(ml_dtypes.bfloat16)

    nc = _build()
    in_maps = []
    for k in range(NCORES):
        sl = slice(k * BL, (k + 1) * BL)
        gidx, w8, e4x, m4, dinv, ohm, tailc, ident = _prep_core(
            words[sl], target[sl], corpus[sl], inputs["shared_W"],
            inputs["shared_b"], inputs["domain_A"], inputs["domain_b"],
            inputs["trans_m"], inputs["start_scores"], inputs["end_scores"],
        )
        in_maps.append({
            "table": table8, "gidx": gidx, "w8": w8.view(np.uint8), "e4x": e4x,
            "m4": m4, "dinv": dinv, "ohm": ohm, "tailc": tailc, "ident": ident,
        })
    res = run_bass_kernel_spmd(
        nc, in_maps, core_ids=list(range(NCORES)), trace=_trace,
    )
    out = np.concatenate([res.results[k]["nll"] for k in range(NCORES)])
    return out.astype(np.float32)


# revision 23
# speedup vs baseline: 1.4440x; 1.1605x over previous
"""Trainium2 Bass kernel for nn_Bert_Proj_CRF (BERT projection + CRF NLL).

Strategy (data-parallel over batch, 8 NeuronCores x 8 sequences):
  - Embedding rows are gathered in fp8 (e3m4, x64 scale) with a transpose
    gather: byte d of a row lands at partition (d//2)%128, free offset
    (d//512, token, d%2).  Weight rows are host-permuted to match, so the
    projection matmul runs directly on the gathered layout (fp8 PE).
  - No softmax: NLL = ln(Z_scan(exp(raw))) - sum(raw[tgt]) + host consts,
    because the per-token log-sum-exp terms cancel between the CRF
    normalizer and the gold score (up to a tiny mask[0] correction).
  - CRF normalizer via a pairwise product tree of 4x4 matrices
    M_t = (exp(trans)*exp(bias_b)/4) * u_t (identity where masked), with
    the /4 keeping magnitudes O(1) (host adds n_unmask*ln4 back).
    Tokens are gathered in bit-reversed lane order so the tree pairs
    partition halves at every level - no transpose DMA round trips.
"""

import numpy as np
import ml_dtypes

import concourse.bass as bass
import concourse.bacc as bacc
import concourse.tile as tile
import concourse.mybir as mybir

V, D, T = 21128, 768, 4
B, S = 64, 512
NCORES = 8
BL = B // NCORES            # 8 sequences per core
NGATH = 8                   # gathers per core (1 sequence each)
TPG = S                     # 512 tokens per gather
SC = 64.0                   # fp8 quantization scale
ISC = 1.0 / (SC * SC)
F32 = mybir.dt.float32
BF16 = mybir.dt.bfloat16
F8 = mybir.dt.float8e3
I16 = mybir.dt.int16
AF = mybir.ActivationFunctionType
AL = mybir.AluOpType
AX = mybir.AxisListType

_REV7 = np.array([int(format(p, "07b")[::-1], 2) for p in range(128)])


def fap(t, off, dims):
    """AP over tile t's partition dim with custom free dims (element units)."""
    base = t if isinstance(t, bass.AP) else t[:]
    return bass.AP(
        tensor=base.tensor,
        offset=base.offset + off,
        ap=[list(base.ap[0])] + [list(d) for d in dims],
    )


def pap(t, p0, p1, off, dims):
    """Like fap but restricted to partitions [p0, p1)."""
    base = t if isinstance(t, bass.AP) else t[:]
    pd = list(base.ap[0])
    return bass.AP(
        tensor=base.tensor,
        offset=base.offset + p0 * pd[0] + off,
        ap=[[pd[0], p1 - p0]] + [list(d) for d in dims],
    )


def dap(handle, ap):
    return bass.AP(tensor=handle, offset=0, ap=[list(d) for d in ap])


_CACHE = {}


def _build():
    if "nc" in _CACHE:
        return _CACHE["nc"]
    nc = bacc.Bacc()

    # fp8 table bytes typed as bf16 (D/2 elems): the 16-bit transpose gather
    # moves byte pairs; matmuls bitcast the gathered tile back to fp8.
    table_h = nc.dram_tensor("table", [V, D // 2], BF16, kind="ExternalInput")
    gidx_h = nc.dram_tensor("gidx", [128, NGATH * TPG // 16], I16, kind="ExternalInput")
    w8_h = nc.dram_tensor("w8", [128, 6 * BL * T], F8, kind="ExternalInput")
    e4x_h = nc.dram_tensor("e4x", [BL * 4 * 16], BF16, kind="ExternalInput")
    m4_h = nc.dram_tensor("m4", [128, BL * 4], BF16, kind="ExternalInput")
    dinv_h = nc.dram_tensor("dinv", [128, BL * 4], BF16, kind="ExternalInput")
    ohm_h = nc.dram_tensor("ohm", [128, BL * 16], BF16, kind="ExternalInput")
    ident_h = nc.dram_tensor("ident", [128, 128], BF16, kind="ExternalInput")
    tailc_h = nc.dram_tensor("tailc", [128], F32, kind="ExternalInput")
    nll_h = nc.dram_tensor("nll", [BL], F32, kind="ExternalOutput")

    with tile.TileContext(nc) as tc:
        with (
            tc.tile_pool(name="consts", bufs=1) as cp,
            tc.tile_pool(name="xt", bufs=NGATH) as xp,
            tc.tile_pool(name="work", bufs=1) as wp,
            tc.tile_pool(name="psum", bufs=1, space="PSUM") as pp,
            tc.tile_pool(name="psum2", bufs=1, space="PSUM") as pp2,
        ):
            # ---- inputs: gidx FIRST so gathers start asap ----
            gidx = cp.tile([128, NGATH * TPG // 16], I16)
            nc.sync.dma_start(out=gidx[:], in_=gidx_h[:])
            w8 = cp.tile([128, 6 * BL * T], F8)
            nc.sync.dma_start(out=w8[:], in_=w8_h[:])
            e4x = cp.tile([128, BL * 4 * 16], BF16)
            nc.sync.dma_start(out=e4x[:], in_=dap(e4x_h, [[0, 128], [1, BL * 4 * 16]]))
            m4 = cp.tile([128, BL * 4], BF16)
            nc.sync.dma_start(out=m4[:], in_=m4_h[:])
            dinv = cp.tile([128, BL * 4], BF16)
            nc.sync.dma_start(out=dinv[:], in_=dinv_h[:])
            ohm = cp.tile([128, BL * 16], BF16)
            nc.sync.dma_start(out=ohm[:], in_=ohm_h[:])
            tailc = cp.tile([1, 128], F32)
            nc.sync.dma_start(out=tailc[:], in_=dap(tailc_h, [[0, 1], [1, 128]]))
            ident = cp.tile([128, 128], BF16)
            nc.sync.dma_start(out=ident[:], in_=ident_h[:])
            ones128 = cp.tile([128, 1], F32)
            nc.vector.memset(ones128[:], 1.0)

            # per-pair-group tiles so each epilogue chain depends only on its
            # own gathers/matmuls and overlaps the remaining gather window
            lgs = [pp.tile([128, 8, T], F32, name=f"lg{i}") for i in range(4)]
            us = [wp.tile([128, 8 * T], BF16, name=f"u{i}") for i in range(4)]
            vs = [wp.tile([128, 8 * T], BF16, name=f"v{i}") for i in range(4)]
            Mfs = [wp.tile([128, 8 * 16], BF16, name=f"Mf{i}") for i in range(4)]
            t1s = [wp.tile([128, 256], BF16, name=f"t1g{i}") for i in range(4)]
            t2s = [wp.tile([128, 128], BF16, name=f"t2g{i}") for i in range(4)]
            P1s = [wp.tile([128, 64], BF16, name=f"P1g{i}") for i in range(4)]
            em = wp.tile([128, BL * 16], F32)        # raw*onehot*mask/4096
            t1 = wp.tile([128, 512], BF16)
            t2 = wp.tile([128, 256], BF16)
            B4a = wp.tile([128, BL * 16], BF16)      # 4-step blocks
            B4b = wp.tile([128, BL * 16], BF16)
            a0 = wp.tile([1, 32], F32)               # u0 * exp(bias+start)
            a0e = wp.tile([1, 32], F32)              # u0 * exp(bias)

            xts = []
            for g in range(NGATH):
                xt = xp.tile([128, 3, TPG], BF16, tag="xt")
                xts.append(xt)
                nc.gpsimd.dma_gather(
                    out_ap=xt[:],
                    in_ap=table_h[:],
                    idxs_ap=gidx[:, g * (TPG // 16):(g + 1) * (TPG // 16)],
                    num_idxs=TPG,
                    num_idxs_reg=TPG,
                    elem_size=D // 2,
                    transpose=True,
                )

            for g in range(4):
                lg, u, v, Mf = lgs[g], us[g], vs[g], Mfs[g]
                t1g, t2g, P1t = t1s[g], t2s[g], P1s[g]
                # ---- projection matmuls (fp8 view) for b = 2g, 2g+1 ----
                for bp in range(2):
                    b = 2 * g + bp
                    xf8 = xts[b][:].bitcast(F8)
                    for gl in range(4):
                        tok0 = gl * 128
                        for cb in range(6):
                            c16, bit = cb // 2, cb % 2
                            lhsT = fap(xf8, c16 * 2 * TPG + tok0 * 2 + bit, [[2, 128]])
                            nc.tensor.matmul(
                                lg[:, bp * 4 + gl, :],
                                lhsT=lhsT,
                                rhs=w8[:, cb * BL * T + b * T:(cb * BL * T + b * T) + T],
                                start=(cb == 0),
                                stop=(cb == 5),
                            )
                # ---- epilogue for this pair of sequences ----
                lg_sl = fap(lg, 0, [[1, 32]])
                nc.scalar.activation(out=u[:], in_=lg_sl, func=AF.Exp, scale=ISC)
                nc.vector.tensor_tensor(
                    out=fap(em, 32 * g, [[1, 32]]), in0=lg_sl,
                    in1=fap(ohm, 32 * g, [[1, 32]]), op=AL.mult,
                )
                nc.vector.tensor_tensor(
                    out=v[:], in0=u[:],
                    in1=fap(m4, 8 * g, [[1, 8], [0, 4]]), op=AL.mult,
                )
                nc.vector.tensor_tensor(
                    out=fap(a0, 8 * g, [[1, 8]]),
                    in0=pap(u, 0, 1, 0, [[16, 2], [1, 4]]),
                    in1=fap(tailc, 8 * g, [[1, 8]]), op=AL.mult,
                )
                nc.vector.tensor_tensor(
                    out=fap(a0e, 8 * g, [[1, 8]]),
                    in0=pap(u, 0, 1, 0, [[16, 2], [1, 4]]),
                    in1=fap(tailc, 64 + 8 * g, [[1, 8]]), op=AL.mult,
                )
                # Mf[:, (b,gl), k, j] = v[j] * E4x[(b,gl),k,j];  diag += dinv
                nc.vector.tensor_tensor(
                    out=Mf[:],
                    in0=fap(v, 0, [[4, 8], [0, 4], [1, 4]]),
                    in1=fap(e4x, 128 * g, [[1, 128]]),
                    op=AL.mult,
                )
                nc.vector.tensor_tensor(
                    out=fap(Mf, 0, [[16, 8], [5, 4]]),
                    in0=fap(Mf, 0, [[16, 8], [5, 4]]),
                    in1=fap(dinv, 8 * g, [[1, 8], [0, 4]]),
                    op=AL.add,
                )
                # ---- within-lane fold L1: (gl0*gl1), (gl2*gl3) ----
                nc.vector.tensor_tensor(
                    out=fap(t1g, 0, [[64, 4], [1, 64]]),
                    in0=fap(Mf, 0, [[32, 4], [1, 16], [0, 4]]),
                    in1=fap(Mf, 16, [[32, 4], [0, 4], [1, 16]]),
                    op=AL.mult,
                )
                nc.vector.tensor_tensor(
                    out=fap(t2g, 0, [[32, 4], [1, 32]]),
                    in0=fap(t1g, 0, [[64, 4], [16, 4], [8, 2], [1, 4]]),
                    in1=fap(t1g, 4, [[64, 4], [16, 4], [8, 2], [1, 4]]),
                    op=AL.add,
                )
                nc.vector.tensor_tensor(
                    out=fap(P1t, 0, [[16, 4], [1, 16]]),
                    in0=fap(t2g, 0, [[32, 4], [8, 4], [1, 4]]),
                    in1=fap(t2g, 4, [[32, 4], [8, 4], [1, 4]]),
                    op=AL.add,
                )
                # ---- L2: per bp, pair products -> B4[:, b, 16] ----
                nc.vector.tensor_tensor(
                    out=fap(t1g, 0, [[64, 2], [1, 64]]),
                    in0=fap(P1t, 0, [[32, 2], [1, 16], [0, 4]]),
                    in1=fap(P1t, 16, [[32, 2], [0, 4], [1, 16]]),
                    op=AL.mult,
                )
                nc.vector.tensor_tensor(
                    out=fap(t2g, 0, [[32, 2], [1, 32]]),
                    in0=fap(t1g, 0, [[64, 2], [16, 4], [8, 2], [1, 4]]),
                    in1=fap(t1g, 4, [[64, 2], [16, 4], [8, 2], [1, 4]]),
                    op=AL.add,
                )
                nc.vector.tensor_tensor(
                    out=fap(B4a, 32 * g, [[16, 2], [1, 16]]),
                    in0=fap(t2g, 0, [[32, 2], [8, 4], [1, 4]]),
                    in1=fap(t2g, 4, [[32, 2], [8, 4], [1, 4]]),
                    op=AL.add,
                )

            # ---- cross-partition tree: 7 levels of halves pairing ----
            # DVE can't read two SBUF operands at different base partitions,
            # so the upper half is first shifted down via a PE matmul with a
            # sliced identity (PSUM operand dodges the equal-base rule).
            psh = pp2.tile([128, BL * 16], F32)
            cur, nxt = B4a, B4b
            for lvl in range(7):
                half = 64 >> lvl
                nc.tensor.matmul(
                    pap(psh, 0, half, 0, [[1, BL * 16]]),
                    lhsT=pap(ident, 0, 2 * half, half, [[1, half]]),
                    rhs=pap(cur, 0, 2 * half, 0, [[1, BL * 16]]),
                    start=True, stop=True,
                )
                nc.vector.tensor_tensor(
                    out=pap(t1, 0, half, 0, [[64, 8], [1, 64]]),
                    in0=pap(cur, 0, half, 0, [[16, 8], [1, 16], [0, 4]]),
                    in1=pap(psh, 0, half, 0, [[16, 8], [0, 4], [1, 16]]),
                    op=AL.mult,
                )
                nc.vector.tensor_tensor(
                    out=pap(t2, 0, half, 0, [[32, 8], [1, 32]]),
                    in0=pap(t1, 0, half, 0, [[64, 8], [16, 4], [8, 2], [1, 4]]),
                    in1=pap(t1, 0, half, 4, [[64, 8], [16, 4], [8, 2], [1, 4]]),
                    op=AL.add,
                )
                nc.vector.tensor_tensor(
                    out=pap(nxt, 0, half, 0, [[16, 8], [1, 16]]),
                    in0=pap(t2, 0, half, 0, [[32, 8], [8, 4], [1, 4]]),
                    in1=pap(t2, 0, half, 4, [[32, 8], [8, 4], [1, 4]]),
                    op=AL.add,
                )
                cur, nxt = nxt, cur

            # ---- tail on partition 0: Z, gold, corrections ----
            sv = fap(tailc, 0, [[1, 32]])       # exp(bias+start) per (b,j)
            ee = fap(tailc, 32, [[1, 32]])      # exp(end) per (b,j)
            eb0 = fap(tailc, 64, [[1, 32]])     # exp(bias) per (b,j)
            hg = fap(tailc, 96, [[1, 8]])       # n_unmask*ln4 - host_gold
            cm0 = fap(tailc, 104, [[1, 8]])     # mask0 - 1

            tz = wp.tile([1, 128], F32)
            nc.vector.tensor_tensor(
                out=tz[:], in0=pap(cur, 0, 1, 0, [[1, 128]]),
                in1=fap(a0, 0, [[4, 8], [1, 4], [0, 4]]),
                op=AL.mult,
            )
            tz2 = wp.tile([1, 64], F32)
            nc.vector.tensor_tensor(
                out=tz2[:], in0=fap(tz, 0, [[16, 8], [8, 2], [1, 4]]),
                in1=fap(tz, 4, [[16, 8], [8, 2], [1, 4]]), op=AL.add,
            )
            za = wp.tile([1, 32], F32)
            nc.vector.tensor_tensor(
                out=za[:], in0=fap(tz2, 0, [[8, 8], [1, 4]]),
                in1=fap(tz2, 4, [[8, 8], [1, 4]]), op=AL.add,
            )
            ze = wp.tile([1, 32], F32)
            nc.vector.tensor_tensor(out=ze[:], in0=za[:], in1=ee, op=AL.mult)
            z8 = wp.tile([1, BL], F32)
            nc.vector.reduce_sum(out=z8[:], in_=fap(ze, 0, [[4, 8], [1, 4]]), axis=AX.X)
            lnz = wp.tile([1, BL], F32)
            nc.scalar.activation(out=lnz[:], in_=z8[:], func=AF.Ln)

            # mask0 correction: (m0-1)*ln(sum_j u0*exp(bias))
            s0 = wp.tile([1, BL], F32)
            nc.vector.reduce_sum(out=s0[:], in_=fap(a0e, 0, [[4, 8], [1, 4]]), axis=AX.X)
            lns0 = wp.tile([1, BL], F32)
            nc.scalar.activation(out=lns0[:], in_=s0[:], func=AF.Ln)
            corr = wp.tile([1, BL], F32)
            nc.vector.tensor_tensor(out=corr[:], in0=lns0[:], in1=cm0, op=AL.mult)

            # gold emit: ones^T @ em -> per-column sums -> per-b
            gold_ps = pp2.tile([1, BL * 16], F32)
            nc.tensor.matmul(gold_ps[:], lhsT=ones128[:], rhs=em[:], start=True, stop=True)
            emit8 = wp.tile([1, BL], F32)
            nc.vector.reduce_sum(
                out=emit8[:], in_=fap(gold_ps, 0, [[16, 8], [1, 16]]), axis=AX.X
            )

            nll = wp.tile([1, BL], F32)
            nc.vector.tensor_tensor(out=nll[:], in0=lnz[:], in1=hg, op=AL.add)
            nc.vector.tensor_tensor(out=nll[:], in0=nll[:], in1=emit8[:], op=AL.subtract)
            nc.vector.tensor_tensor(out=nll[:], in0=nll[:], in1=corr[:], op=AL.add)
            nc.sync.dma_start(out=nll_h[:], in_=nll[:])

    nc.compile()
    _CACHE["nc"] = nc
    return nc


def _prep_core(words, target, corpus, shared_W, shared_b, domain_A, domain_b,
               trans_m, start_scores, end_scores):
    w = np.asarray(words, np.int64)
    t = np.asarray(target, np.int64)
    sw = np.asarray(shared_W, np.float32)
    sb = np.asarray(shared_b, np.float32)
    dA = np.asarray(domain_A, np.float32)
    db = np.asarray(domain_b, np.float32)
    tm = np.asarray(trans_m, np.float32)
    ss = np.asarray(start_scores, np.float32)
    es = np.asarray(end_scores, np.float32)

    rev = _REV7
    # token s for (b, gl, p): s = 4*rev7(p) + gl
    s_of = (4 * rev[None, :] + np.arange(4)[:, None, None]).reshape(1, 4, 128)  # (1,gl,p)

    # gidx: gather g = sequence g, flat order (gl, p).  For 768B rows the
    # engine reads the 16-wide index wrap from partitions 16..31 (measured).
    gidx = np.zeros((128, NGATH * TPG // 16), np.int16)
    for g in range(NGATH):
        seq = np.empty(TPG, np.int64)
        for gl in range(4):
            seq[gl * 128:gl * 128 + 128] = w[g][4 * rev + gl]
        gidx[16:32, g * (TPG // 16):(g + 1) * (TPG // 16)] = (
            seq.reshape(TPG // 16, 16).T.astype(np.int16))

    W = sw[None] + dA[corpus]                      # (BL, D, T)
    bias = sb[None] + db[corpus]                   # (BL, T)
    W8q = np.asarray((W * SC).astype(ml_dtypes.float8_e3m4))
    # w8[p, cb, b, j] = W8q[b, 2*((cb//2)*128+p) + cb%2, j]
    cb = np.arange(6)
    p = np.arange(128)
    drow = 2 * ((cb[None, :] // 2) * 128 + p[:, None]) + (cb[None, :] % 2)  # (128, 6)
    w8 = np.ascontiguousarray(
        W8q[:, drow, :].transpose(1, 2, 0, 3).reshape(128, 6 * BL * T))

    eT = np.exp(tm)                                # (4,4) k,j
    e4 = (eT[None, :, :] * np.exp(bias)[:, None, :] / 4.0)   # (BL, k, j)
    e4x = np.ascontiguousarray(
        np.broadcast_to(e4[:, None, :, :], (BL, 4, 4, 4)).reshape(-1)
    ).astype(ml_dtypes.bfloat16)

    mask = (w != 0)                                # (BL, S)
    m = mask.astype(np.float32)
    # scan mask per (p, b, gl): step s = 4*rev(p)+gl, zero at s==0
    sm = np.zeros((128, BL, 4), np.float32)
    for gl in range(4):
        s_idx = 4 * rev + gl                       # (128,)
        sm[:, :, gl] = m[:, s_idx].T
        if gl == 0:
            sm[rev == 0, :, 0] = 0.0               # s==0 -> identity
    m4 = np.ascontiguousarray(sm.reshape(128, BL * 4)).astype(ml_dtypes.bfloat16)
    dinv = np.ascontiguousarray(1.0 - sm.reshape(128, BL * 4)).astype(ml_dtypes.bfloat16)

    # ohm[p, b, gl, j] = (target==j)*(mask)*ISC at s = 4*rev(p)+gl
    ohm = np.zeros((128, BL, 4, 4), np.float32)
    for gl in range(4):
        s_idx = 4 * rev + gl
        oh = (np.eye(4, dtype=np.float32)[t[:, s_idx]] * m[:, s_idx, None])  # (BL,128,4)
        ohm[:, :, gl, :] = oh.transpose(1, 0, 2)
    ohm = np.ascontiguousarray((ohm * ISC).reshape(128, BL * 16)).astype(ml_dtypes.bfloat16)

    bidx = np.arange(BL)
    tr = tm[t[:, :-1], t[:, 1:]] * m[:, 1:]
    last_idx = np.maximum(m.sum(1).astype(np.int64) - 1, 0)
    host_gold = ((bias[bidx[:, None], t] * m).sum(1) + tr.sum(1)
                 + ss[t[:, 0]] + es[t[bidx, last_idx]])
    n_unmask = m[:, 1:].sum(1)

    tailc = np.zeros(128, np.float32)
    tailc[0:32] = np.exp(bias + ss[None, :]).reshape(-1)
    tailc[32:64] = np.tile(np.exp(es), BL)
    tailc[64:96] = np.exp(bias).reshape(-1)
    tailc[96:104] = n_unmask * np.log(4.0) - host_gold
    tailc[104:112] = m[:, 0] - 1.0
    ident = np.eye(128, dtype=ml_dtypes.bfloat16)
    return gidx, w8, e4x, m4, dinv, ohm, tailc, ident


def kernel(_trace=False, **inputs):
    from concourse.bass_utils import run_bass_kernel_spmd

    words = np.asarray(inputs["words"])
    target = np.asarray(inputs["target"])
    corpus = np.asarray(inputs["corpus"])
    table8 = np.asarray(
        (np.asarray(inputs["embed_table"], np.float32) * SC).astype(ml_dtypes.float8_e3m4)
    ).view(ml_dtypes.bfloat16)

    nc = _build()
    in_maps = []
    for k in range(NCORES):
        sl = slice(k * BL, (k + 1) * BL)
        gidx, w8, e4x, m4, dinv, ohm, tailc, ident = _prep_core(
            words[sl], target[sl], corpus[sl], inputs["shared_W"],
            inputs["shared_b"], inputs["domain_A"], inputs["domain_b"],
            inputs["trans_m"], inputs["start_scores"], inputs["end_scores"],
        )
        in_maps.append({
            "table": table8, "gidx": gidx, "w8": w8.view(np.uint8), "e4x": e4x,
            "m4": m4, "dinv": dinv, "ohm": ohm, "tailc": tailc, "ident": ident,
        })
    res = run_bass_kernel_spmd(
        nc, in_maps, core_ids=list(range(NCORES)), trace=_trace,
    )
    out = np.concatenate([res.results[k]["nll"] for k in range(NCORES)])
    return out.astype(np.float32)


# revision 26
# speedup vs baseline: 1.4577x; 1.0095x over previous
"""Trainium2 Bass kernel for nn_Bert_Proj_CRF (BERT projection + CRF NLL).

Strategy (data-parallel over batch, 8 NeuronCores x 8 sequences):
  - Embedding rows are gathered in fp8 (e3m4, x64 scale) with a transpose
    gather: byte d of a row lands at partition (d//2)%128, free offset
    (d//512, token, d%2).  Weight rows are host-permuted to match, so the
    projection matmul runs directly on the gathered layout (fp8 PE).
  - No softmax: NLL = ln(Z_scan(exp(raw))) - sum(raw[tgt]) + host consts,
    because the per-token log-sum-exp terms cancel between the CRF
    normalizer and the gold score (up to a tiny mask[0] correction).
  - CRF normalizer via a pairwise product tree of 4x4 matrices
    M_t = (exp(trans)*exp(bias_b)/4) * u_t (identity where masked), with
    the /4 keeping magnitudes O(1) (host adds n_unmask*ln4 back).
    Tokens are gathered in bit-reversed lane order so the tree pairs
    partition halves at every level - no transpose DMA round trips.
"""

import numpy as np
import ml_dtypes

import concourse.bass as bass
import concourse.bacc as bacc
import concourse.tile as tile
import concourse.mybir as mybir

V, D, T = 21128, 768, 4
B, S = 64, 512
NCORES = 8
BL = B // NCORES            # 8 sequences per core
NGATH = 8                   # gathers per core (1 sequence each)
TPG = S                     # 512 tokens per gather
SC = 64.0                   # fp8 quantization scale
ISC = 1.0 / (SC * SC)
F32 = mybir.dt.float32
BF16 = mybir.dt.bfloat16
F8 = mybir.dt.float8e3
I16 = mybir.dt.int16
AF = mybir.ActivationFunctionType
AL = mybir.AluOpType
AX = mybir.AxisListType

_REV7 = np.array([int(format(p, "07b")[::-1], 2) for p in range(128)])


def fap(t, off, dims):
    """AP over tile t's partition dim with custom free dims (element units)."""
    base = t if isinstance(t, bass.AP) else t[:]
    return bass.AP(
        tensor=base.tensor,
        offset=base.offset + off,
        ap=[list(base.ap[0])] + [list(d) for d in dims],
    )


def pap(t, p0, p1, off, dims):
    """Like fap but restricted to partitions [p0, p1)."""
    base = t if isinstance(t, bass.AP) else t[:]
    pd = list(base.ap[0])
    return bass.AP(
        tensor=base.tensor,
        offset=base.offset + p0 * pd[0] + off,
        ap=[[pd[0], p1 - p0]] + [list(d) for d in dims],
    )


def dap(handle, ap):
    return bass.AP(tensor=handle, offset=0, ap=[list(d) for d in ap])


_CACHE = {}


def _build():
    if "nc" in _CACHE:
        return _CACHE["nc"]
    nc = bacc.Bacc()

    # fp8 table bytes typed as bf16 (D/2 elems): the 16-bit transpose gather
    # moves byte pairs; matmuls bitcast the gathered tile back to fp8.
    table_h = nc.dram_tensor("table", [V, D // 2], BF16, kind="ExternalInput")
    gidx_h = nc.dram_tensor("gidx", [128, NGATH * TPG // 16], I16, kind="ExternalInput")
    w8_h = nc.dram_tensor("w8", [128, 6 * BL * T], F8, kind="ExternalInput")
    e4x_h = nc.dram_tensor("e4x", [BL * 4 * 16], BF16, kind="ExternalInput")
    m4_h = nc.dram_tensor("m4", [128, BL * 4], BF16, kind="ExternalInput")
    dinv_h = nc.dram_tensor("dinv", [128, BL * 4], BF16, kind="ExternalInput")
    ohm_h = nc.dram_tensor("ohm", [128, BL * 16], BF16, kind="ExternalInput")
    ident_h = nc.dram_tensor("ident", [128, 128], BF16, kind="ExternalInput")
    tailc_h = nc.dram_tensor("tailc", [128], F32, kind="ExternalInput")
    nll_h = nc.dram_tensor("nll", [BL], F32, kind="ExternalOutput")

    with tile.TileContext(nc) as tc:
        with (
            tc.tile_pool(name="consts", bufs=1) as cp,
            tc.tile_pool(name="xt", bufs=NGATH) as xp,
            tc.tile_pool(name="work", bufs=1) as wp,
            tc.tile_pool(name="psum", bufs=1, space="PSUM") as pp,
            tc.tile_pool(name="psum2", bufs=1, space="PSUM") as pp2,
        ):
            # ---- inputs: gidx FIRST so gathers start asap ----
            # gidx via the Pool queue: SWDGE desc-gen starts immediately and
            # the gathers (also on Pool) chain right behind it in-order
            gidx = cp.tile([128, NGATH * TPG // 16], I16)
            nc.gpsimd.dma_start(out=gidx[:], in_=gidx_h[:])
            w8 = cp.tile([128, 6 * BL * T], F8)
            nc.sync.dma_start(out=w8[:], in_=w8_h[:])
            e4x = cp.tile([128, BL * 4 * 16], BF16)
            nc.sync.dma_start(out=e4x[:], in_=dap(e4x_h, [[0, 128], [1, BL * 4 * 16]]))
            m4 = cp.tile([128, BL * 4], BF16)
            nc.sync.dma_start(out=m4[:], in_=m4_h[:])
            dinv = cp.tile([128, BL * 4], BF16)
            nc.sync.dma_start(out=dinv[:], in_=dinv_h[:])
            ohm = cp.tile([128, BL * 16], BF16)
            nc.sync.dma_start(out=ohm[:], in_=ohm_h[:])
            tailc = cp.tile([1, 128], F32)
            nc.sync.dma_start(out=tailc[:], in_=dap(tailc_h, [[0, 1], [1, 128]]))
            ident = cp.tile([128, 128], BF16)
            nc.sync.dma_start(out=ident[:], in_=ident_h[:])
            ones128 = cp.tile([128, 1], F32)
            nc.vector.memset(ones128[:], 1.0)

            # per-pair-group tiles so each epilogue chain depends only on its
            # own gathers/matmuls and overlaps the remaining gather window
            lgs = [pp.tile([128, 8, T], F32, name=f"lg{i}") for i in range(4)]
            us = [wp.tile([128, 8 * T], BF16, name=f"u{i}") for i in range(4)]
            vs = [wp.tile([128, 8 * T], BF16, name=f"v{i}") for i in range(4)]
            Mfs = [wp.tile([128, 8 * 16], BF16, name=f"Mf{i}") for i in range(4)]
            t1s = [wp.tile([128, 256], BF16, name=f"t1g{i}") for i in range(4)]
            t2s = [wp.tile([128, 128], BF16, name=f"t2g{i}") for i in range(4)]
            P1s = [wp.tile([128, 64], BF16, name=f"P1g{i}") for i in range(4)]
            em = wp.tile([128, BL * 16], F32)        # raw*onehot*mask/4096
            t1 = wp.tile([128, 512], BF16)
            t2 = wp.tile([128, 256], BF16)
            B4a = wp.tile([128, BL * 16], BF16)      # 4-step blocks
            B4b = wp.tile([128, BL * 16], BF16)
            a0 = wp.tile([1, 32], F32)               # u0 * exp(bias+start)
            a0e = wp.tile([1, 32], F32)              # u0 * exp(bias)

            xts = []
            for g in range(NGATH):
                xt = xp.tile([128, 3, TPG], BF16, tag="xt")
                xts.append(xt)
                nc.gpsimd.dma_gather(
                    out_ap=xt[:],
                    in_ap=table_h[:],
                    idxs_ap=gidx[:, g * (TPG // 16):(g + 1) * (TPG // 16)],
                    num_idxs=TPG,
                    num_idxs_reg=TPG,
                    elem_size=D // 2,
                    transpose=True,
                )

            for g in range(4):
                lg, u, v, Mf = lgs[g], us[g], vs[g], Mfs[g]
                t1g, t2g, P1t = t1s[g], t2s[g], P1s[g]
                # ---- projection matmuls (fp8 view) for b = 2g, 2g+1 ----
                for bp in range(2):
                    b = 2 * g + bp
                    xf8 = xts[b][:].bitcast(F8)
                    for gl in range(4):
                        tok0 = gl * 128
                        for cb in range(6):
                            c16, bit = cb // 2, cb % 2
                            lhsT = fap(xf8, c16 * 2 * TPG + tok0 * 2 + bit, [[2, 128]])
                            nc.tensor.matmul(
                                lg[:, bp * 4 + gl, :],
                                lhsT=lhsT,
                                rhs=w8[:, cb * BL * T + b * T:(cb * BL * T + b * T) + T],
                                start=(cb == 0),
                                stop=(cb == 5),
                            )
                # ---- epilogue for this pair of sequences ----
                lg_sl = fap(lg, 0, [[1, 32]])
                nc.scalar.activation(out=u[:], in_=lg_sl, func=AF.Exp, scale=ISC)
                nc.vector.tensor_tensor(
                    out=fap(em, 32 * g, [[1, 32]]), in0=lg_sl,
                    in1=fap(ohm, 32 * g, [[1, 32]]), op=AL.mult,
                )
                nc.vector.tensor_tensor(
                    out=v[:], in0=u[:],
                    in1=fap(m4, 8 * g, [[1, 8], [0, 4]]), op=AL.mult,
                )
                nc.vector.tensor_tensor(
                    out=fap(a0, 8 * g, [[1, 8]]),
                    in0=pap(u, 0, 1, 0, [[16, 2], [1, 4]]),
                    in1=fap(tailc, 8 * g, [[1, 8]]), op=AL.mult,
                )
                nc.vector.tensor_tensor(
                    out=fap(a0e, 8 * g, [[1, 8]]),
                    in0=pap(u, 0, 1, 0, [[16, 2], [1, 4]]),
                    in1=fap(tailc, 64 + 8 * g, [[1, 8]]), op=AL.mult,
                )
                # Mf[:, (b,gl), k, j] = v[j] * E4x[(b,gl),k,j];  diag += dinv
                nc.vector.tensor_tensor(
                    out=Mf[:],
                    in0=fap(v, 0, [[4, 8], [0, 4], [1, 4]]),
                    in1=fap(e4x, 128 * g, [[1, 128]]),
                    op=AL.mult,
                )
                nc.vector.tensor_tensor(
                    out=fap(Mf, 0, [[16, 8], [5, 4]]),
                    in0=fap(Mf, 0, [[16, 8], [5, 4]]),
                    in1=fap(dinv, 8 * g, [[1, 8], [0, 4]]),
                    op=AL.add,
                )
                # ---- within-lane fold L1: (gl0*gl1), (gl2*gl3) ----
                nc.vector.tensor_tensor(
                    out=fap(t1g, 0, [[64, 4], [1, 64]]),
                    in0=fap(Mf, 0, [[32, 4], [1, 16], [0, 4]]),
                    in1=fap(Mf, 16, [[32, 4], [0, 4], [1, 16]]),
                    op=AL.mult,
                )
                nc.vector.tensor_tensor(
                    out=fap(t2g, 0, [[32, 4], [1, 32]]),
                    in0=fap(t1g, 0, [[64, 4], [16, 4], [8, 2], [1, 4]]),
                    in1=fap(t1g, 4, [[64, 4], [16, 4], [8, 2], [1, 4]]),
                    op=AL.add,
                )
                nc.vector.tensor_tensor(
                    out=fap(P1t, 0, [[16, 4], [1, 16]]),
                    in0=fap(t2g, 0, [[32, 4], [8, 4], [1, 4]]),
                    in1=fap(t2g, 4, [[32, 4], [8, 4], [1, 4]]),
                    op=AL.add,
                )
                # ---- L2: per bp, pair products -> B4[:, b, 16] ----
                nc.vector.tensor_tensor(
                    out=fap(t1g, 0, [[64, 2], [1, 64]]),
                    in0=fap(P1t, 0, [[32, 2], [1, 16], [0, 4]]),
                    in1=fap(P1t, 16, [[32, 2], [0, 4], [1, 16]]),
                    op=AL.mult,
                )
                nc.vector.tensor_tensor(
                    out=fap(t2g, 0, [[32, 2], [1, 32]]),
                    in0=fap(t1g, 0, [[64, 2], [16, 4], [8, 2], [1, 4]]),
                    in1=fap(t1g, 4, [[64, 2], [16, 4], [8, 2], [1, 4]]),
                    op=AL.add,
                )
                nc.vector.tensor_tensor(
                    out=fap(B4a, 32 * g, [[16, 2], [1, 16]]),
                    in0=fap(t2g, 0, [[32, 2], [8, 4], [1, 4]]),
                    in1=fap(t2g, 4, [[32, 2], [8, 4], [1, 4]]),
                    op=AL.add,
                )

            # ---- cross-partition tree: 7 levels of halves pairing ----
            # DVE can't read two SBUF operands at different base partitions,
            # so the upper half is first shifted down via a PE matmul with a
            # sliced identity (PSUM operand dodges the equal-base rule).
            psh = pp2.tile([128, BL * 16], F32)
            cur, nxt = B4a, B4b
            for lvl in range(7):
                half = 64 >> lvl
                nc.tensor.matmul(
                    pap(psh, 0, half, 0, [[1, BL * 16]]),
                    lhsT=pap(ident, 0, 2 * half, half, [[1, half]]),
                    rhs=pap(cur, 0, 2 * half, 0, [[1, BL * 16]]),
                    start=True, stop=True,
                )
                nc.vector.tensor_tensor(
                    out=pap(t1, 0, half, 0, [[64, 8], [1, 64]]),
                    in0=pap(cur, 0, half, 0, [[16, 8], [1, 16], [0, 4]]),
                    in1=pap(psh, 0, half, 0, [[16, 8], [0, 4], [1, 16]]),
                    op=AL.mult,
                )
                nc.vector.tensor_tensor(
                    out=pap(t2, 0, half, 0, [[32, 8], [1, 32]]),
                    in0=pap(t1, 0, half, 0, [[64, 8], [16, 4], [8, 2], [1, 4]]),
                    in1=pap(t1, 0, half, 4, [[64, 8], [16, 4], [8, 2], [1, 4]]),
                    op=AL.add,
                )
                nc.vector.tensor_tensor(
                    out=pap(nxt, 0, half, 0, [[16, 8], [1, 16]]),
                    in0=pap(t2, 0, half, 0, [[32, 8], [8, 4], [1, 4]]),
                    in1=pap(t2, 0, half, 4, [[32, 8], [8, 4], [1, 4]]),
                    op=AL.add,
                )
                cur, nxt = nxt, cur

            # ---- tail on partition 0: Z, gold, corrections ----
            sv = fap(tailc, 0, [[1, 32]])       # exp(bias+start) per (b,j)
            ee = fap(tailc, 32, [[1, 32]])      # exp(end) per (b,j)
            eb0 = fap(tailc, 64, [[1, 32]])     # exp(bias) per (b,j)
            hg = fap(tailc, 96, [[1, 8]])       # n_unmask*ln4 - host_gold
            cm0 = fap(tailc, 104, [[1, 8]])     # mask0 - 1

            tz = wp.tile([1, 128], F32)
            nc.vector.tensor_tensor(
                out=tz[:], in0=pap(cur, 0, 1, 0, [[1, 128]]),
                in1=fap(a0, 0, [[4, 8], [1, 4], [0, 4]]),
                op=AL.mult,
            )
            tz2 = wp.tile([1, 64], F32)
            nc.vector.tensor_tensor(
                out=tz2[:], in0=fap(tz, 0, [[16, 8], [8, 2], [1, 4]]),
                in1=fap(tz, 4, [[16, 8], [8, 2], [1, 4]]), op=AL.add,
            )
            za = wp.tile([1, 32], F32)
            nc.vector.tensor_tensor(
                out=za[:], in0=fap(tz2, 0, [[8, 8], [1, 4]]),
                in1=fap(tz2, 4, [[8, 8], [1, 4]]), op=AL.add,
            )
            ze = wp.tile([1, 32], F32)
            nc.vector.tensor_tensor(out=ze[:], in0=za[:], in1=ee, op=AL.mult)
            # z sums and mask0-correction sums side by side -> single Ln
            zs = wp.tile([1, 2 * BL], F32)
            nc.vector.reduce_sum(out=fap(zs, 0, [[1, 8]]),
                                 in_=fap(ze, 0, [[4, 8], [1, 4]]), axis=AX.X)
            nc.vector.reduce_sum(out=fap(zs, 8, [[1, 8]]),
                                 in_=fap(a0e, 0, [[4, 8], [1, 4]]), axis=AX.X)
            lzs = wp.tile([1, 2 * BL], F32)
            nc.scalar.activation(out=lzs[:], in_=zs[:], func=AF.Ln)
            lnz = fap(lzs, 0, [[1, 8]])
            corr = wp.tile([1, BL], F32)
            nc.vector.tensor_tensor(out=corr[:], in0=fap(lzs, 8, [[1, 8]]),
                                    in1=cm0, op=AL.mult)

            # gold emit: ones^T @ em -> per-column sums -> per-b
            gold_ps = pp2.tile([1, BL * 16], F32)
            nc.tensor.matmul(gold_ps[:], lhsT=ones128[:], rhs=em[:], start=True, stop=True)
            emit8 = wp.tile([1, BL], F32)
            nc.vector.reduce_sum(
                out=emit8[:], in_=fap(gold_ps, 0, [[16, 8], [1, 16]]), axis=AX.X
            )

            nll = wp.tile([1, BL], F32)
            nc.vector.tensor_tensor(out=nll[:], in0=lnz, in1=hg, op=AL.add)
            nc.vector.tensor_tensor(out=nll[:], in0=nll[:], in1=emit8[:], op=AL.subtract)
            nc.vector.tensor_tensor(out=nll[:], in0=nll[:], in1=corr[:], op=AL.add)
            nc.sync.dma_start(out=nll_h[:], in_=nll[:])

    nc.compile()
    _CACHE["nc"] = nc
    return nc


def _prep_core(words, target, corpus, shared_W, shared_b, domain_A, domain_b,
               trans_m, start_scores, end_scores):
    w = np.asarray(words, np.int64)
    t = np.asarray(target, np.int64)
    sw = np.asarray(shared_W, np.float32)
    sb = np.asarray(shared_b, np.float32)
    dA = np.asarray(domain_A, np.float32)
    db = np.asarray(domain_b, np.float32)
    tm = np.asarray(trans_m, np.float32)
    ss = np.asarray(start_scores, np.float32)
    es = np.asarray(end_scores, np.float32)

    rev = _REV7
    # token s for (b, gl, p): s = 4*rev7(p) + gl
    s_of = (4 * rev[None, :] + np.arange(4)[:, None, None]).reshape(1, 4, 128)  # (1,gl,p)

    # gidx: gather g = sequence g, flat order (gl, p).  For 768B rows the
    # engine reads the 16-wide index wrap from partitions 16..31 (measured).
    gidx = np.zeros((128, NGATH * TPG // 16), np.int16)
    for g in range(NGATH):
        seq = np.empty(TPG, np.int64)
        for gl in range(4):
            seq[gl * 128:gl * 128 + 128] = w[g][4 * rev + gl]
        gidx[16:32, g * (TPG // 16):(g + 1) * (TPG // 16)] = (
            seq.reshape(TPG // 16, 16).T.astype(np.int16))

    W = sw[None] + dA[corpus]                      # (BL, D, T)
    bias = sb[None] + db[corpus]                   # (BL, T)
    W8q = np.asarray((W * SC).astype(ml_dtypes.float8_e3m4))
    # w8[p, cb, b, j] = W8q[b, 2*((cb//2)*128+p) + cb%2, j]
    cb = np.arange(6)
    p = np.arange(128)
    drow = 2 * ((cb[None, :] // 2) * 128 + p[:, None]) + (cb[None, :] % 2)  # (128, 6)
    w8 = np.ascontiguousarray(
        W8q[:, drow, :].transpose(1, 2, 0, 3).reshape(128, 6 * BL * T))

    eT = np.exp(tm)                                # (4,4) k,j
    e4 = (eT[None, :, :] * np.exp(bias)[:, None, :] / 4.0)   # (BL, k, j)
    e4x = np.ascontiguousarray(
        np.broadcast_to(e4[:, None, :, :], (BL, 4, 4, 4)).reshape(-1)
    ).astype(ml_dtypes.bfloat16)

    mask = (w != 0)                                # (BL, S)
    m = mask.astype(np.float32)
    # scan mask per (p, b, gl): step s = 4*rev(p)+gl, zero at s==0
    sm = np.zeros((128, BL, 4), np.float32)
    for gl in range(4):
        s_idx = 4 * rev + gl                       # (128,)
        sm[:, :, gl] = m[:, s_idx].T
        if gl == 0:
            sm[rev == 0, :, 0] = 0.0               # s==0 -> identity
    m4 = np.ascontiguousarray(sm.reshape(128, BL * 4)).astype(ml_dtypes.bfloat16)
    dinv = np.ascontiguousarray(1.0 - sm.reshape(128, BL * 4)).astype(ml_dtypes.bfloat16)

    # ohm[p, b, gl, j] = (target==j)*(mask)*ISC at s = 4*rev(p)+gl
    ohm = np.zeros((128, BL, 4, 4), np.float32)
    for gl in range(4):
        s_idx = 4 * rev + gl
        oh = (np.eye(4, dtype=np.float32)[t[:, s_idx]] * m[:, s_idx, None])  # (BL,128,4)
        ohm[:, :, gl, :] = oh.transpose(1, 0, 2)
    ohm = np.ascontiguousarray((ohm * ISC).reshape(128, BL * 16)).astype(ml_dtypes.bfloat16)

    bidx = np.arange(BL)
    tr = tm[t[:, :-1], t[:, 1:]] * m[:, 1:]
    last_idx = np.maximum(m.sum(1).astype(np.int64) - 1, 0)
    host_gold = ((bias[bidx[:, None], t] * m).sum(1) + tr.sum(1)
                 + ss[t[:, 0]] + es[t[bidx, last_idx]])
    n_unmask = m[:, 1:].sum(1)

    tailc = np.zeros(128, np.float32)
    tailc[0:32] = np.exp(bias + ss[None, :]).reshape(-1)
    tailc[32:64] = np.tile(np.exp(es), BL)
    tailc[64:96] = np.exp(bias).reshape(-1)
    tailc[96:104] = n_unmask * np.log(4.0) - host_gold
    tailc[104:112] = m[:, 0] - 1.0
    ident = np.eye(128, dtype=ml_dtypes.bfloat16)
    return gidx, w8, e4x, m4, dinv, ohm, tailc, ident


def kernel(_trace=False, **inputs):
    from concourse.bass_utils import run_bass_kernel_spmd

    words = np.asarray(inputs["words"])
    target = np.asarray(inputs["target"])
    corpus = np.asarray(inputs["corpus"])
    table8 = np.asarray(
        (np.asarray(inputs["embed_table"], np.float32) * SC).astype(ml_dtypes.float8_e3m4)
    ).view(ml_dtypes.bfloat16)

    nc = _build()
    in_maps = []
    for k in range(NCORES):
        sl = slice(k * BL, (k + 1) * BL)
        gidx, w8, e4x, m4, dinv, ohm, tailc, ident = _prep_core(
            words[sl], target[sl], corpus[sl], inputs["shared_W"],
            inputs["shared_b"], inputs["domain_A"], inputs["domain_b"],
            inputs["trans_m"], inputs["start_scores"], inputs["end_scores"],
        )
        in_maps.append({
            "table": table8, "gidx": gidx, "w8": w8.view(np.uint8), "e4x": e4x,
            "m4": m4, "dinv": dinv, "ohm": ohm, "tailc": tailc, "ident": ident,
        })
    res = run_bass_kernel_spmd(
        nc, in_maps, core_ids=list(range(NCORES)), trace=_trace,
    )
    out = np.concatenate([res.results[k]["nll"] for k in range(NCORES)])
    return out.astype(np.float32)
